# revision 19
# baseline (speedup 1.0000x reference)
"""Trainium2 Bass kernel v4: 3x3x64->1 valid conv over (512, 512, 64), fp8.

out[r, c] = sum_{fi,fj,d} x[r+fi, c+fj, d] * W[0, (fi*3+fj)*64+d] + b[0]
Output: (510*510,) float32.

Strategy (8-way row sharding, 64 output rows per core + 2-row halo):
  x ships as float8_e3m4 (1 B/elem, rel err ~1.2e-2 vs the 2e-2 gate).
  The cost model charges a matmul only for its MOVING free size, so x
  tiles are the STATIONARY:
    per (row-pair, col-chunk, fj): out[c', r-window] +=
        x_pair[(rho,d), 128c+fj+c']^T @ M_fj[(rho,d), j]
  with M_fj[rho*64+d, j] = w[rho+2-j, fj, d] (bf16) the moving tensor:
  ~400 matmuls of free size <= 4.  Input DMAs are split across the three
  DMA-capable engines (SP, Activation, Pool/SWDGE) which the cost model
  serializes independently, tripling effective DMA issue bandwidth.  The
  M matrix rides as raw bytes at the head of SP's first tensor and is
  read through a bf16 bitcast.  PSUM rows [0,48) (bank A) and [48,64)
  (bank B) are separate banks so DVE can drain A while PE still
  accumulates B (concurrent read of an in-flight bank wedges hardware);
  bias is preloaded by the DVE memsets.  One output DMA on SP at the
  end.  All sync hand-rolled.
"""

from contextlib import ExitStack

import numpy as np
import ml_dtypes

import concourse.bass as bass
import concourse.mybir as mybir
from concourse.bass_utils import run_bass_kernel_spmd

N_CORES = 8
H = 512
WD = 512
D = 64
NOUT = 510
R_PER_CORE = 64           # output rows per core (last 2 of core 7 discarded)
ROWS_IN = R_PER_CORE + 2  # input rows per core incl. halo
NPAIRS = ROWS_IN // 2     # 33

CW = [128, 128, 128, 126]             # output-column chunk widths
ROWS_A = 48                           # rows finalized early (bank A)
PAIR_A = 24                           # A complete once pairs <= 24 are in
MCOLS = 28                            # M bytes in xsp0 (12 bf16 cols + 2 zero)

# (engine, name, [pair ids]) in planned arrival order; 'm' = M prefix
GROUPS = [
    ("sp",   "xsp0",  [0, 1, 2, 3]),
    ("act",  "xact0", [4, 5, 6, 7]),
    ("pool", "xpl0",  [8, 9, 10, 11]),
    ("act",  "xact1", [12, 13, 14, 15, 16]),
    ("pool", "xpl1",  [17, 18, 19, 20]),
    ("sp",   "xsp1",  [21, 22, 23, 24, 25]),
    ("pool", "xpl2",  [26, 27, 28]),
    ("act",  "xact2", [29, 30]),
    ("sp",   "xsp2",  [31, 32]),
]

F8 = mybir.dt.float8e3
U8 = mybir.dt.uint8
BF16 = mybir.dt.bfloat16
F32 = mybir.dt.float32

assert sorted(p for _, _, ps in GROUPS for p in ps) == list(range(NPAIRS))


def _build_nc(bias_val: float) -> bass.Bass:
    nc = bass.Bass()
    dram = {}
    for g, (eng, name, pairs) in enumerate(GROUPS):
        cols = len(pairs) * WD + (MCOLS if g == 0 else 0)
        dram[name] = nc.dram_tensor(name, [128, cols], U8, kind="ExternalInput")
    out_dram = nc.dram_tensor("out", [128, 256], F32, kind="ExternalOutput")

    with ExitStack() as ctx:
        sb = {}
        for g, (eng, name, pairs) in enumerate(GROUPS):
            cols = len(pairs) * WD + (MCOLS if g == 0 else 0)
            sb[name] = ctx.enter_context(
                nc.sbuf_tensor(name + "s", [128, cols], U8))
        osb = ctx.enter_context(nc.sbuf_tensor("osb", [128, 256], F32))
        acc_a = ctx.enter_context(nc.psum_tensor("acc_a", [128, 4 * ROWS_A], F32))
        acc_b = ctx.enter_context(
            nc.psum_tensor("acc_b", [128, 4 * (R_PER_CORE - ROWS_A)], F32))

        gsem = [ctx.enter_context(nc.semaphore(f"g{g}"))
                for g in range(len(GROUPS))]
        zm_sem = ctx.enter_context(nc.semaphore("zm_sem"))
        pe_sem = ctx.enter_context(nc.semaphore("pe_sem"))
        ca_sem = ctx.enter_context(nc.semaphore("ca_sem"))
        cb_sem = ctx.enter_context(nc.semaphore("cb_sem"))
        out_sem = ctx.enter_context(nc.semaphore("out_sem"))
        block = ctx.enter_context(nc.Block())

        def issue(engine_handle, which):
            for g, (eng, name, pairs) in enumerate(GROUPS):
                if eng == which:
                    engine_handle.dma_start(sb[name][:, :], dram[name][:, :]) \
                        .then_inc(gsem[g], 16)

        @block.sync
        def _(sync):
            issue(sync, "sp")
            sync.wait_ge(ca_sem, 1)
            sync.wait_ge(cb_sem, 1)
            sync.dma_start(out_dram[:, :], osb[:, :]).then_inc(out_sem, 16)

        @block.scalar
        def _(scalar):
            issue(scalar, "act")

        @block.gpsimd
        def _(gpsimd):
            issue(gpsimd, "pool")

        @block.vector
        def _(vector):
            nc.vector.memset(acc_a[:, :], 0.0).then_inc(zm_sem, 1)
            nc.vector.memset(acc_b[:, :], 0.0).then_inc(zm_sem, 1)
            vector.wait_ge(pe_sem, 1)
            nc.vector.tensor_scalar_add(osb[:, 0:4 * ROWS_A], acc_a[:, :],
                                        float(bias_val)).then_inc(ca_sem, 1)
            vector.wait_ge(pe_sem, 2)
            nc.vector.tensor_scalar_add(osb[:, 4 * ROWS_A:256], acc_b[:, :],
                                        float(bias_val)).then_inc(cb_sem, 1)

        @block.tensor
        def _(tensor):
            tensor.wait_ge(zm_sem, 2)
            rb = R_PER_CORE - ROWS_A
            m_sb = sb[GROUPS[0][1]]

            def acc_ap(c, cw, lo, hi):
                if hi <= ROWS_A:
                    return acc_a[0:cw, ROWS_A * c + lo: ROWS_A * c + hi]
                return acc_b[0:cw, rb * c + lo - ROWS_A: rb * c + hi - ROWS_A]

            for g, (eng, name, pairs) in enumerate(GROUPS):
                tensor.wait_ge(gsem[g], 16)
                for l, j in enumerate(pairs):
                    r0 = 2 * j
                    wlo, whi = max(0, r0 - 2), min(R_PER_CORE, r0 + 2)
                    if wlo < ROWS_A < whi:
                        spans = [(wlo, ROWS_A), (ROWS_A, whi)]
                    else:
                        spans = [(wlo, whi)]
                    for c in range(4):
                        cw = CW[c]
                        base = (MCOLS if g == 0 else 0) + l * WD + 128 * c
                        for fj in range(3):
                            for lo, hi in spans:
                                mlo = lo - (r0 - 2)
                                mhi = hi - (r0 - 2)
                                mm = nc.tensor.matmul(
                                    acc_ap(c, cw, lo, hi),
                                    lhsT=sb[name][:, base + fj:
                                                 base + fj + cw].bitcast(F8),
                                    rhs=m_sb[:, 2 * (4 * fj + mlo):
                                             2 * (4 * fj + mhi)].bitcast(BF16),
                                    start=False, stop=False,
                                    skip_group_check=True,
                                )
                    # spacer matmuls: accumulate the two zero bf16 M columns
                    # into the just-finished bank, so the pe_sem increment
                    # rides an instruction issued after the real writes have
                    # drained (real HW retires the sem before PSUM settles)
                    if j == PAIR_A:
                        nc.tensor.matmul(
                            acc_a[0:128, 4 * ROWS_A - 2: 4 * ROWS_A],
                            lhsT=m_sb[:, MCOLS:MCOLS + 128].bitcast(F8),
                            rhs=m_sb[:, 24:28].bitcast(BF16),
                            start=False, stop=False,
                            skip_group_check=True).then_inc(pe_sem, 1)
                    if j == GROUPS[-1][2][-1]:
                        nc.tensor.matmul(
                            acc_b[0:128, 4 * rb - 2: 4 * rb],
                            lhsT=m_sb[:, MCOLS:MCOLS + 128].bitcast(F8),
                            rhs=m_sb[:, 24:28].bitcast(BF16),
                            start=False, stop=False,
                            skip_group_check=True).then_inc(pe_sem, 1)

    return nc


def _prep_inputs(x: np.ndarray, W: np.ndarray):
    xt = np.ascontiguousarray(x.transpose(0, 2, 1))  # (512, 64, 512)
    xt_pad = np.zeros((N_CORES * R_PER_CORE + 2, D, WD), np.float32)
    xt_pad[:H] = xt
    x8 = xt_pad.astype(ml_dtypes.float8_e3m4)

    w = np.asarray(W, np.float32)[0].reshape(3, 3, D)
    # M[rho*64+d, 4*fj+j] = w[rho+2-j, fj, d]; out row r = r0-2+j
    # cols 12-13 stay zero: rhs for the drain-spacer matmuls
    M = np.zeros((128, 14), np.float32)
    for rho in range(2):
        for fj in range(3):
            for jcol in range(4):
                fi = rho + 2 - jcol
                if 0 <= fi < 3:
                    M[rho * 64:(rho + 1) * 64, 4 * fj + jcol] = w[fi, fj]
    Mb = np.ascontiguousarray(M.astype(ml_dtypes.bfloat16))
    M8 = Mb.view(ml_dtypes.float8_e3m4)  # [128, 28] raw bytes

    in_maps = []
    for i in range(N_CORES):
        shard = x8[R_PER_CORE * i: R_PER_CORE * i + ROWS_IN]
        pairs_arr = shard.reshape(NPAIRS, 2, D, WD).transpose(1, 2, 0, 3) \
                         .reshape(128, NPAIRS * WD)
        m = {}
        for g, (eng, name, pairs) in enumerate(GROUPS):
            blocks = [pairs_arr[:, j * WD: (j + 1) * WD] for j in pairs]
            if g == 0:
                blocks.insert(0, M8)
            m[name] = np.ascontiguousarray(
                np.concatenate(blocks, axis=1)).view(np.uint8)
        in_maps.append(m)
    return in_maps


def kernel(x: np.ndarray, W: np.ndarray, b: np.ndarray, _trace=False):
    x = np.asarray(x, np.float32)
    in_maps = _prep_inputs(x, W)
    nc = _build_nc(float(np.asarray(b).reshape(-1)[0]))
    res = run_bass_kernel_spmd(nc, in_maps, core_ids=list(range(N_CORES)),
                               trace=_trace)
    full = np.zeros((N_CORES * R_PER_CORE, 512), np.float32)
    rb = R_PER_CORE - ROWS_A
    for i in range(N_CORES):
        o = res.results[i]["out"]                      # [128, 256]
        A = o[:, :4 * ROWS_A].reshape(128, 4, ROWS_A)  # [c', c, r<48]
        B = o[:, 4 * ROWS_A:].reshape(128, 4, rb)      # [c', c, r-48]
        for c in range(4):
            cw = CW[c]
            full[R_PER_CORE * i: R_PER_CORE * i + ROWS_A,
                 128 * c: 128 * c + cw] = A[0:cw, c, :].T
            full[R_PER_CORE * i + ROWS_A: R_PER_CORE * (i + 1),
                 128 * c: 128 * c + cw] = B[0:cw, c, :].T
    out = full[:NOUT, :NOUT].reshape(-1).astype(np.float32)
    if _trace:
        return out, res
    return out
